# revision 17
# baseline (speedup 1.0000x reference)
"""Trainium2 Bass kernel for nn_DistanceLoss (5-way episodic cosine-distance loss).

Math (reference): S=[25,80,512], Q=[200,80,512] row-normalized; sim[s,i,q,j] =
Sn[s,i]*Qn[q,j]; fro2[s,q] = sum_ij (1-sim)^2; logits[q,c] =
-mean_{s in class c} 2*fro2[s,q].

Identity used: fro2 = F^2 - 2*(u_s.v_q) + SS[s,q], with u_s = sum_i Sn[s,i],
v_q = sum_j Qn[q,j], SS = sum_ij sim^2. Only SS needs the full sim matrix;
sum_ij sim collapses to a rank-1 term. Class-mean and the 2x fold into
host-built weight matrices, so logits = 2*UVc - SSc - 2*F^2.

Sharding: queries split across 8 cores (25 each); support replicated.
Row layout: all global rows first (r=16s+i), then all local rows
(r=400+64s+j) so the HBM loads are contiguous; the host-built selection
matrices use the same layout.
"""

import sys

sys.path.insert(0, "/opt/trn_rl_repo")

import numpy as np
import ml_dtypes

import concourse.bass as bass
import concourse.tile as tile
from concourse import mybir
from concourse.bass_utils import run_bass_kernel_spmd
import bass_rust as _bass_rust

NS = 25          # support count
NQ = 200         # total queries
NCORES = 8
NQC = NQ // NCORES   # queries per core
FG, FL = 16, 64
F = FG + FL      # 80 rows per item
D = 512
WAY = 5
ROWS = NS * F    # 2000 support rows; also NQC * F = 2000 query rows per core
GROWS = NS * FG  # 400 (same for the per-core query slice)
NSTRIP = (ROWS + 127) // 128   # 16 strips of <=128 rows
ROWSP = NSTRIP * 128   # 2048: consts padded so the strip-major DMA view works
NCHUNK = 4                     # sim column chunks
CW = ROWS // NCHUNK            # 500 columns per chunk (PSUM-bank limit 512 f32)
BF16 = mybir.dt.bfloat16
F32 = mybir.dt.float32


def _strip_rows(t):
    lo = 128 * t
    hi = min(lo + 128, ROWS)
    return lo, hi - lo


def _emit_strip(nc, pools, g_dram, l_dram, xT, ident_bf, t):
    """Load one 128-row strip, row-normalize to bf16, transpose into the
    four persistent [128, ROWS] d-chunk tiles. Returns the natural tile."""
    ld, small, natp, scr, trps = pools
    tag = g_dram.name[:1]
    lo, pr = _strip_rows(t)

    raw = ld.tile([128, D], F32, name=f"raw_{tag}")
    # rows < GROWS come from the global tensor, the rest from local
    if lo + pr <= GROWS:
        nc.sync.dma_start(out=raw[:pr], in_=g_dram[lo : lo + pr, :])
    elif lo >= GROWS:
        nc.sync.dma_start(out=raw[:pr], in_=l_dram[lo - GROWS : lo - GROWS + pr, :])
    else:
        ng = GROWS - lo
        nc.sync.dma_start(out=raw[:ng], in_=g_dram[lo:GROWS, :])
        nc.sync.dma_start(out=raw[ng:pr], in_=l_dram[0 : pr - ng, :])

    nrm2 = small.tile([128, 1], F32, name=f"nrm2_{tag}")
    dump = scr.tile([128, D], BF16, name=f"dump_{tag}")
    nc.vector.scalar_tensor_tensor(
        out=dump[:pr],
        in0=raw[:pr],
        scalar=0.0,
        in1=raw[:pr],
        op0=mybir.AluOpType.bypass,
        op1=mybir.AluOpType.mult,
        accum_out=nrm2[:pr],
    )
    nrm = small.tile([128, 1], F32, name=f"nrm_{tag}")
    nc.scalar.sqrt(nrm[:pr], nrm2[:pr])
    rnrm = small.tile([128, 1], F32, name=f"rnrm_{tag}")
    nc.vector.reciprocal(rnrm[:pr], nrm[:pr])

    nat = natp.tile([128, D], BF16, name=f"nat_{tag}")
    nc.vector.tensor_scalar_mul(nat[:pr], raw[:pr], rnrm[:pr])

    for k in range(4):
        tr = trps.tile([128, 128], BF16, name=f"tr_{tag}", tag="tr")
        nc.tensor.transpose(
            tr[:, :pr], nat[:pr, 128 * k : 128 * (k + 1)], ident_bf[:pr, :pr]
        )
        # alternate the PSUM->SBUF copies between DVE and ACT
        if (t + k) % 2 == 0:
            nc.vector.tensor_copy(out=xT[k][:, lo : lo + pr], in_=tr[:, :pr])
        else:
            nc.scalar.copy(xT[k][:, lo : lo + pr], tr[:, :pr])
    return nat


def _build_program():
    nc = bass.Bass()

    s_g = nc.dram_tensor("s_g", [GROWS, D], F32, kind="ExternalInput")
    s_l = nc.dram_tensor("s_l", [ROWS - GROWS, D], F32, kind="ExternalInput")
    q_g = nc.dram_tensor("q_g", [GROWS, D], F32, kind="ExternalInput")
    q_l = nc.dram_tensor("q_l", [ROWS - GROWS, D], F32, kind="ExternalInput")
    wsel_d = nc.dram_tensor("wsel", [ROWSP, WAY], BF16, kind="ExternalInput")
    esel_d = nc.dram_tensor("esel", [ROWSP, NQC], BF16, kind="ExternalInput")
    mrow_d = nc.dram_tensor("mrow", [ROWSP, WAY], F32, kind="ExternalInput")
    id_bf_d = nc.dram_tensor("id_bf", [128, 128], BF16, kind="ExternalInput")
    id_f32_d = nc.dram_tensor("id_f32", [128, 128], F32, kind="ExternalInput")
    logits_d = nc.dram_tensor("logits", [NQC, WAY], F32, kind="ExternalOutput")

    with tile.TileContext(nc) as tc:
        with (
            tc.tile_pool(name="ld", bufs=6) as ld,
            tc.tile_pool(name="small", bufs=8) as small,
            tc.tile_pool(name="natp", bufs=16) as natp,
            tc.tile_pool(name="scr", bufs=3) as scr,
            tc.tile_pool(name="persist", bufs=1) as persist,
            tc.tile_pool(name="sq", bufs=3) as sqp,
        ):
            # constants
            ident_bf = persist.tile([128, 128], BF16, name="ident_bf")
            nc.sync.dma_start(out=ident_bf, in_=id_bf_d[:])
            ident_f32 = persist.tile([128, 128], F32, name="ident_f32")
            nc.sync.dma_start(out=ident_f32, in_=id_f32_d[:])
            wsel = persist.tile([128, NSTRIP, WAY], BF16, name="wsel")
            nc.sync.dma_start(
                out=wsel, in_=wsel_d[:].rearrange("(t p) c -> p t c", p=128)
            )
            esel = persist.tile([128, NSTRIP, NQC], BF16, name="esel")
            nc.sync.dma_start(
                out=esel, in_=esel_d[:].rearrange("(t p) c -> p t c", p=128)
            )
            mrow = persist.tile([128, NSTRIP, WAY], F32, name="mrow")
            nc.sync.dma_start(
                out=mrow, in_=mrow_d[:].rearrange("(t p) c -> p t c", p=128)
            )

            ST = [persist.tile([128, ROWS], BF16, name=f"sT_{k}") for k in range(4)]
            QT = [persist.tile([128, ROWS], BF16, name=f"qT_{k}") for k in range(4)]

            with (
                tc.tile_pool(name="trps", bufs=3, space="PSUM") as trps,
                tc.tile_pool(name="accps", bufs=1, space="PSUM") as accps,
            ):
                pools = (ld, small, natp, scr, trps)
                # interleave S and Q strips so early sim chunks unblock fast
                natS, natQ = [], []
                for t in range(NSTRIP):
                    natQ.append(_emit_strip(nc, pools, q_g, q_l, QT, ident_bf, t))
                    natS.append(_emit_strip(nc, pools, s_g, s_l, ST, ident_bf, t))

                # class-weighted support row-sums ucT[c,d] and per-query
                # row-sums vT[q,d], accumulated over strips on PE
                ucT_ps = accps.tile([WAY, D], F32, name="ucT_ps")
                vT_ps = accps.tile([NQC, D], F32, name="vT_ps")
                for t in range(NSTRIP):
                    lo, pr = _strip_rows(t)
                    nc.tensor.matmul(
                        ucT_ps[:, :],
                        wsel[:pr, t, :],
                        natS[t][:pr, :],
                        start=(t == 0),
                        stop=(t == NSTRIP - 1),
                        skip_group_check=True,
                    )
                    nc.tensor.matmul(
                        vT_ps[:, :],
                        esel[:pr, t, :],
                        natQ[t][:pr, :],
                        start=(t == 0),
                        stop=(t == NSTRIP - 1),
                        skip_group_check=True,
                    )
                ucT_sb = persist.tile([WAY, D], F32, name="ucT_sb")
                nc.vector.tensor_copy(out=ucT_sb, in_=ucT_ps)
                vT_sb = persist.tile([NQC, D], F32, name="vT_sb")
                nc.vector.tensor_copy(out=vT_sb, in_=vT_ps)

                # transpose ucT/vT into [d,c]/[d,q] chunks for the final matmul
                uc_sb, v_sb = [], []
                for k in range(4):
                    tru = trps.tile([128, 128], F32, name="tru", tag="trf", bufs=2)
                    nc.tensor.transpose(
                        tru[:, :WAY],
                        ucT_sb[:, 128 * k : 128 * (k + 1)],
                        ident_f32[:WAY, :WAY],
                    )
                    uck = persist.tile([128, WAY], F32, name=f"uc_{k}")
                    nc.vector.tensor_copy(out=uck, in_=tru[:, :WAY])
                    uc_sb.append(uck)

                    trv = trps.tile([128, 128], F32, name="trv", tag="trf", bufs=2)
                    nc.tensor.transpose(
                        trv[:, :NQC],
                        vT_sb[:, 128 * k : 128 * (k + 1)],
                        ident_f32[:NQC, :NQC],
                    )
                    vk = persist.tile([128, NQC], F32, name=f"v_{k}")
                    nc.vector.tensor_copy(out=vk, in_=trv[:, :NQC])
                    v_sb.append(vk)

            # ---- sim + squared block-reduction ----
            with (
                tc.tile_pool(name="simps", bufs=6, space="PSUM") as simps,
                tc.tile_pool(name="finps", bufs=1, space="PSUM") as finps,
            ):
                ssrow = []
                for t in range(NSTRIP):
                    ssrow.append(persist.tile([128, NQC], F32, name=f"ssrow_{t}"))
                for t in range(NSTRIP):
                    lo, pr = _strip_rows(t)
                    # k-outer: the 4 column chunks accumulate in parallel PSUM
                    sims = [
                        simps.tile([128, CW], F32, name="sim") for _ in range(NCHUNK)
                    ]
                    for k in range(4):
                        for n in range(NCHUNK):
                            nc.tensor.matmul(
                                sims[n][:pr, :],
                                ST[k][:, lo : lo + pr],
                                QT[k][:, CW * n : CW * (n + 1)],
                                start=(k == 0),
                                stop=(k == 3),
                                skip_group_check=True,
                            )
                    sq = sqp.tile([128, ROWS], BF16, name="sq")
                    for n in range(NCHUNK):
                        nc.scalar.square(sq[:pr, CW * n : CW * (n + 1)], sims[n][:pr, :])
                    nc.vector.tensor_reduce(
                        out=ssrow[t][:pr, :],
                        in_=sq[:pr, :].rearrange("p (b j) -> p b j", j=F),
                        axis=mybir.AxisListType.X,
                        op=mybir.AluOpType.add,
                    )

                # SSc[q,c] = sum_rows ssrow[row,q] * mrow[row,c]
                ss_ps = finps.tile([NQC, WAY], F32, name="ss_ps")
                for t in range(NSTRIP):
                    lo, pr = _strip_rows(t)
                    nc.tensor.matmul(
                        ss_ps[:, :],
                        ssrow[t][:pr, :],
                        mrow[:pr, t, :],
                        start=(t == 0),
                        stop=(t == NSTRIP - 1),
                        skip_group_check=True,
                    )
                # UVc[q,c] = sum_d v[d,q] * uc[d,c]
                uv_ps = finps.tile([NQC, WAY], F32, name="uv_ps")
                for k in range(4):
                    nc.tensor.matmul(
                        uv_ps[:, :],
                        v_sb[k][:, :],
                        uc_sb[k][:, :],
                        start=(k == 0),
                        stop=(k == 3),
                        skip_group_check=True,
                    )

                ss_sb = persist.tile([NQC, WAY], F32, name="ss_sb")
                nc.scalar.copy(ss_sb, ss_ps[:, :])
                out_sb = persist.tile([NQC, WAY], F32, name="out_sb")
                nc.vector.scalar_tensor_tensor(
                    out=out_sb,
                    in0=uv_ps[:, :],
                    scalar=2.0,
                    in1=ss_sb[:, :],
                    op0=mybir.AluOpType.mult,
                    op1=mybir.AluOpType.subtract,
                )
                nc.vector.tensor_scalar_add(out_sb, out_sb, -2.0 * F * F)
                nc.sync.dma_start(out=logits_d[:], in_=out_sb)

    # this walrus build allows at most 1 sync wait per instruction; split
    # multi-wait instructions into standalone event-semaphore waits
    _bass_rust.generate_event_semaphores(nc)
    return nc


_NC_CACHE = None


def _host_consts(support_labels):
    labels = np.asarray(support_labels).astype(np.int64)
    cnt = np.bincount(labels, minlength=WAY).astype(np.float64)
    V = np.zeros((NS, WAY), np.float64)
    V[np.arange(NS), labels] = 2.0 / cnt[labels]

    s_of_row = np.empty(ROWS, np.int64)
    s_of_row[:GROWS] = np.arange(GROWS) // FG
    s_of_row[GROWS:] = np.arange(ROWS - GROWS) // FL
    q_of_row = s_of_row  # same layout for the query side

    pad = ROWSP - ROWS
    wsel = np.pad(V[s_of_row], ((0, pad), (0, 0))).astype(ml_dtypes.bfloat16)
    mrow = np.pad(V[s_of_row], ((0, pad), (0, 0))).astype(np.float32)
    esel = np.pad(
        (q_of_row[:, None] == np.arange(NQC)[None, :]).astype(np.float64),
        ((0, pad), (0, 0)),
    ).astype(ml_dtypes.bfloat16)
    ident = np.eye(128)
    return {
        "wsel": wsel,
        "esel": esel,
        "mrow": mrow,
        "id_bf": ident.astype(ml_dtypes.bfloat16),
        "id_f32": ident.astype(np.float32),
    }


def kernel(
    support_set_global,
    support_set_local,
    support_labels,
    queries_global,
    queries_local,
):
    global _NC_CACHE
    if _NC_CACHE is None:
        _NC_CACHE = _build_program()
    nc = _NC_CACHE

    consts = _host_consts(support_labels)
    s_g = np.ascontiguousarray(
        np.asarray(support_set_global, np.float32).reshape(GROWS, D)
    )
    s_l = np.ascontiguousarray(
        np.asarray(support_set_local, np.float32).reshape(ROWS - GROWS, D)
    )
    qg = np.asarray(queries_global, np.float32)
    ql = np.asarray(queries_local, np.float32)

    in_maps = []
    for c in range(NCORES):
        sl = slice(c * NQC, (c + 1) * NQC)
        in_maps.append(
            dict(
                s_g=s_g,
                s_l=s_l,
                q_g=np.ascontiguousarray(qg[sl].reshape(GROWS, D)),
                q_l=np.ascontiguousarray(ql[sl].reshape(ROWS - GROWS, D)),
                **consts,
            )
        )

    res = run_bass_kernel_spmd(nc, in_maps, core_ids=list(range(NCORES)))
    out = np.concatenate([res.results[c]["logits"] for c in range(NCORES)], axis=0)
    return out.astype(np.float32)
